# revision 28
# baseline (speedup 1.0000x reference)
"""Trainium2 Bass kernel for nn_ContinuousEmbedding (masked matmul + bias).

Computes out = x @ (weights * mask) + bias, reshaped to [B, in_size, out_size],
where mask zeroes each input feature's own [out_size]-wide diagonal block.

Strategy: tensor-parallel across the 8 NeuronCores by splitting the
in_size*out_size (=16384) output columns into 8 shards of 2048 columns.
The rel-err budget (2e-2) is large, so all matmul I/O is bf16: inputs are
cast on the host, the PE runs bf16 at full rate, and the output shard is
stored to HBM as bf16 (halving the dominant store traffic) then upcast on
the host.

Compute orientation is TRANSPOSED vs the torch view: each core computes
out_t[col, batch], i.e. matmul with lhsT = W[k, col_block] (stationary)
and rhs = x^T[k, batch] (moving).  That puts the io-columns on PSUM
partitions, so the bias becomes a per-partition scalar — eviction is a
1-op fused add+cast via tensor_scalar (DVE) / activation-Identity (ACT),
alternating between the two engines so eviction keeps up with the PE.
The host transposes the gathered [2048, 4096] shards back to [B, io].

All inputs (bias, masked W shard, x^T) are packed on the host into ONE
[128, 12304] bf16 "blob" laid out in exact consumption order, so each
load chunk is a single DMA with fat (2-8 KB) per-partition contiguous
descriptors and one completion semaphore.  The first 772 KB chunk is
everything needed to start compute.

Mask is constant — folded into the weights on the host.
"""

import numpy as np

B = 4096
IN_SIZE = 256
OUT_SIZE = 64
IO = IN_SIZE * OUT_SIZE          # 16384
N_CORES = 8
N_SHARD = IO // N_CORES          # 2048 output columns per core
P = 128                          # SBUF/PSUM partitions
KO = IN_SIZE // P                # 2 contraction sub-tiles
M_BLOCKS = N_SHARD // P          # 16 col-blocks per core
N_TILE = 512                     # matmul moving free dim (fp32 PSUM bank)
G_TILE = 1024                    # eviction group width (2 PSUM banks)
G_PER_M = B // G_TILE            # 4 groups per col-block
PSUM_BUFS = 4                    # 4 x 2 banks = all 8 PSUM banks
INTER = 4                        # col-blocks processed group-major first
WARM_MM = 52                     # PE warm-up matmuls (HAM un-throttle)

# ---- blob column layout (bf16 elements, consumption order) ----
OFF_BIAS = 0                                  # [0, 16)   bias_sw[p, m]
OFF_W0 = 16                                   # W k-half x m0..3: 2 x 512
OFF_XT_G0 = OFF_W0 + KO * INTER * P           # 1040: xt g0, 2 k-halves x 1024
OFF_XT_G1 = OFF_XT_G0 + KO * G_TILE           # 3088: xt g1
OFF_XT_G23 = OFF_XT_G1 + KO * G_TILE          # 5136: xt g2-3, 2 x 2048
OFF_W1 = OFF_XT_G23 + KO * 2 * G_TILE         # 9232: W m4..15, 2 x 1536
TOTAL = OFF_W1 + KO * (M_BLOCKS - INTER) * P  # 12304


def _w_off(k, m):
    if m < INTER:
        return OFF_W0 + k * INTER * P + m * P
    return OFF_W1 + k * (M_BLOCKS - INTER) * P + (m - INTER) * P


def _xt_off(k, n):
    if n < G_TILE:
        return OFF_XT_G0 + k * G_TILE + n
    if n < 2 * G_TILE:
        return OFF_XT_G1 + k * G_TILE + (n - G_TILE)
    return OFF_XT_G23 + k * 2 * G_TILE + (n - 2 * G_TILE)


_CACHE: dict = {}


def _build_program():
    import concourse.mybir as mybir
    import concourse.tile as tile
    from concourse import bacc

    nc = bacc.Bacc(
        "TRN2", target_bir_lowering=False, debug=False, num_devices=N_CORES
    )
    bf16 = mybir.dt.bfloat16
    f32 = mybir.dt.float32
    blob = nc.dram_tensor("blob", [P, TOTAL], bf16, kind="ExternalInput").ap()
    # transposed output shard: out_t[col, batch]
    out = nc.dram_tensor("out", [N_SHARD, B], bf16, kind="ExternalOutput").ap()

    with tile.TileContext(nc) as tc:
        with tc.tile_pool(name="const", bufs=1) as const, \
             tc.tile_pool(name="psum", bufs=PSUM_BUFS, space="PSUM") as psum_pool, \
             tc.tile_pool(name="outp", bufs=6) as outp:
            blob_sb = const.tile([P, TOTAL], bf16)

            # Loads in consumption order on a single HWDGE ring; chunk 1 is
            # everything the first 16 groups of matmuls need.
            ld = nc.sync
            for lo, hi in [(0, OFF_XT_G1), (OFF_XT_G1, OFF_XT_G23),
                           (OFF_XT_G23, OFF_W1), (OFF_W1, TOTAL)]:
                ld.dma_start(out=blob_sb[:, lo:hi], in_=blob[:, lo:hi])

            # Warm-up while inputs stream in: short dummy matmuls keep the
            # PE busy until the first chunk lands so the HAM clock-gate is
            # at 8/8 (full rate) for the whole real stream; a dummy
            # activation pulls the ACT function table in early.
            warm_w = const.tile([P, P], bf16)
            warmf = const.tile([1, 1], f32)
            nc.vector.memset(warm_w, 0.0)
            nc.vector.memset(warmf, 0.0)
            nc.scalar.add(warmf[:], warmf[:], warmf[0:1, 0:1])
            # Unpack the packed bf16 bias columns to f32 (DVE scalar
            # operands must be f32).
            bias_sb = const.tile([P, M_BLOCKS], f32)
            nc.vector.tensor_copy(bias_sb[:], blob_sb[:, 0:M_BLOCKS])
            warm_ps = psum_pool.tile([P, G_TILE], f32, name="warm_ps", tag="ps")
            for _ in range(WARM_MM):
                nc.tensor.matmul(
                    warm_ps[:, 0:P], lhsT=warm_w[:], rhs=warm_w[:],
                    start=True, stop=True,
                )

            # Execution order: group-major over the first INTER col-blocks
            # (so full x^T is only needed after ~16 groups), then
            # block-major for the rest.
            order = [(m, g) for g in range(G_PER_M) for m in range(INTER)]
            order += [(m, g) for m in range(INTER, M_BLOCKS)
                      for g in range(G_PER_M)]
            out_sbs = {}
            for pos, (m, g) in enumerate(order):
                ms = slice(m * P, (m + 1) * P)
                if m not in out_sbs:
                    out_sbs[m] = outp.tile([P, B], bf16, name=f"osb{m}",
                                           tag="osb")
                out_sb = out_sbs[m]
                ps = psum_pool.tile([P, G_TILE], f32, name=f"ps{m}_{g}",
                                    tag="ps")
                for k in range(KO):
                    wof = _w_off(k, m)
                    for s in range(G_TILE // N_TILE):
                        n0 = g * G_TILE + s * N_TILE
                        xof = _xt_off(k, n0)
                        nc.tensor.matmul(
                            ps[:, s * N_TILE:(s + 1) * N_TILE],
                            lhsT=blob_sb[:, wof:wof + P],
                            rhs=blob_sb[:, xof:xof + N_TILE],
                            start=(k == 0),
                            stop=(k == KO - 1),
                        )
                gs = slice(g * G_TILE, (g + 1) * G_TILE)
                last = (m >= M_BLOCKS - 2)
                if last:
                    # Last two col-blocks: split every eviction across both
                    # engines so each group clears in ~0.7us (shorter ops →
                    # no end-of-stream queue backlog), and store the final
                    # group per half so the last store starts ASAP.
                    h = G_TILE // 2
                    nc.vector.tensor_scalar_add(
                        out_sb[:, gs.start:gs.start + h],
                        ps[:, 0:h], bias_sb[:, m:m + 1]
                    )
                    nc.scalar.add(
                        out_sb[:, gs.start + h:gs.stop],
                        ps[:, h:], bias_sb[:, m:m + 1]
                    )
                    if m == M_BLOCKS - 1 and g == G_PER_M - 1:
                        ld.dma_start(out=out[ms, gs.start:gs.start + h],
                                     in_=out_sb[:, gs.start:gs.start + h])
                        ld.dma_start(out=out[ms, gs.start + h:gs.stop],
                                     in_=out_sb[:, gs.start + h:gs.stop])
                    elif m == M_BLOCKS - 1:
                        ld.dma_start(out=out[ms, gs], in_=out_sb[:, gs])
                    elif g % 2 == 1:
                        hs = slice((g - 1) * G_TILE, (g + 1) * G_TILE)
                        ld.dma_start(out=out[ms, hs], in_=out_sb[:, hs])
                    continue
                if pos % 2 == 0:
                    nc.vector.tensor_scalar_add(
                        out_sb[:, gs], ps[:], bias_sb[:, m:m + 1]
                    )
                else:
                    nc.scalar.add(out_sb[:, gs], ps[:], bias_sb[:, m:m + 1])
                # Stores per batch-half (512 KB each, smooth stream).
                if g % 2 == 1:
                    hs = slice((g - 1) * G_TILE, (g + 1) * G_TILE)
                    ld.dma_start(out=out[ms, hs], in_=out_sb[:, hs])

    nc.compile()
    return nc


def _get_program():
    if "prog" not in _CACHE:
        _CACHE["prog"] = _build_program()
    return _CACHE["prog"]


def _shard_inputs(x, weights, bias):
    import ml_dtypes

    bf16 = ml_dtypes.bfloat16
    # Fold the constant block-diagonal mask into the weights on the host.
    col_block = np.arange(IO, dtype=np.int64) // OUT_SIZE
    mask = (col_block[None, :] != np.arange(IN_SIZE)[:, None])
    wm = (weights * mask.astype(weights.dtype)).astype(bf16)
    xt = np.ascontiguousarray(x.T.astype(bf16))
    in_maps = []
    for c in range(N_CORES):
        sl = slice(c * N_SHARD, (c + 1) * N_SHARD)
        ws = wm[:, sl]
        blob = np.empty((P, TOTAL), dtype=bf16)
        blob[:, 0:M_BLOCKS] = bias[sl].reshape(M_BLOCKS, P).T.astype(bf16)
        blob[:, M_BLOCKS:OFF_W0] = 0
        for k in range(KO):
            kr = slice(k * P, (k + 1) * P)
            blob[:, _w_off(k, 0):_w_off(k, 0) + INTER * P] = \
                ws[kr, 0:INTER * P]
            blob[:, _w_off(k, INTER):_w_off(k, INTER) +
                 (M_BLOCKS - INTER) * P] = ws[kr, INTER * P:]
            blob[:, _xt_off(k, 0):_xt_off(k, 0) + G_TILE] = \
                xt[kr, 0:G_TILE]
            blob[:, _xt_off(k, G_TILE):_xt_off(k, G_TILE) + G_TILE] = \
                xt[kr, G_TILE:2 * G_TILE]
            blob[:, _xt_off(k, 2 * G_TILE):_xt_off(k, 2 * G_TILE) +
                 2 * G_TILE] = xt[kr, 2 * G_TILE:B]
        in_maps.append({"blob": blob})
    return in_maps


def run_sharded(in_maps, **kwargs):
    """Run the SPMD program on cores 0-7. kwargs forwarded (e.g. trace)."""
    from concourse.bass_utils import run_bass_kernel_spmd

    nc = _get_program()
    return run_bass_kernel_spmd(
        nc, in_maps, core_ids=list(range(N_CORES)), **kwargs
    )


def kernel(x: np.ndarray, weights: np.ndarray, bias: np.ndarray) -> np.ndarray:
    x = np.asarray(x, dtype=np.float32)
    weights = np.asarray(weights, dtype=np.float32)
    bias = np.asarray(bias, dtype=np.float32)
    in_maps = _shard_inputs(x, weights, bias)
    res = run_sharded(in_maps)
    full = np.empty((B, IO), dtype=np.float32)
    for c in range(N_CORES):
        sl = slice(c * N_SHARD, (c + 1) * N_SHARD)
        full[:, sl] = np.asarray(res.results[c]["out"]).astype(np.float32).T
    return full.reshape(B, IN_SIZE, OUT_SIZE)
